# revision 54
# baseline (speedup 1.0000x reference)
"""GQA attention decode step (B=32, S=1, H=32, KVH=8, D=128, HID=4096, T=2048)
on 8 Trainium2 NeuronCores, tensor-parallel over heads.

Sharding: core i owns query heads 4i..4i+3 and kv head i. Each core: QKV
proj (x @ w shards) -> per-head RMSNorm + RoPE -> attention over its
kv-head's 2048-entry cache (all 32 batches) -> Megatron-style PARTIAL
o_proj against the 512-wide COLUMN shard of wo. No collective: the host
sums the 8 [32,4096] f32 partials (that is the unshard step).

Precision: fp8 e3m4 for wq/wk (x128, absorbed by the scale-invariant q/k
RMSNorms) AND both caches (x24; K's scale folds into the softmax exp()
input scale, V's into the normalization diag). The NEW token's k/v stay
exact bf16: its score column accumulates over the fp8 matmul's zeroed
column (host zeroes the stale slot), and its AV term is a separate K=1
matmul at partition 0 against v_row0 (host ROLLS chunk 15's stream order
to [2047, 1920..2046] so t=2047 sits at partition 0 / pT row 0). Softmax
runs WITHOUT max subtraction (|score*INVS| <= ~13 << 88 = f32 exp
overflow). ~25.8MB HBM/core vs 89MB for the fp32 baseline.

Schedule: ONE sync-ring DMA stream in exact consumption order (x, wq, wk,
wv, rope, kt pairs, v, wo, out) -- order within a ring is the only
reliable arrival-order control, and pool WAR gates are sized away
(kt/v bufs=16). The two score halves run as CONCURRENT col-tiled PE
streams (half 0 in array cols 0-63, half 1 in 64-127, each pulling its
own kt tile), softmax is one merged full-128-row pass (ACT/DVE ops are
free-dim bound), AV packs 4 batches per PSUM bank via tile_position,
flushes write attn-out head-major (oT[d, 32h+b] = contiguous o_proj
lhsT chunks), and o_proj is 4x col-tiled (two [128,512] PSUM tiles hold
all 8 output banks). Throwaway "warmth" matmuls gated on the softmax
exps keep the PE's HAM clock at 2.4GHz through the exp/DVE window.
"""

import sys

sys.path.insert(0, "/opt/trn_rl_repo")

import numpy as np
import ml_dtypes

import concourse.bass as bass
import concourse.tile as tile
from concourse import bacc, mybir
from concourse.bass import ts
from concourse.bass_utils import run_bass_kernel_spmd
from concourse.masks import make_identity

F32 = mybir.dt.float32
BF16 = mybir.dt.bfloat16
FP8 = mybir.dt.float8e3
AF = mybir.ActivationFunctionType
ALU = mybir.AluOpType
AX = mybir.AxisListType

NP_BF16 = ml_dtypes.bfloat16
NP_FP8 = ml_dtypes.float8_e3m4

# Only the K cache rides in fp8 (e3m4, x24 so values sit in the normal
# range [0.25, 15.5]); everything touching the NEW token's k/v stays bf16 --
# the fresh RMSNormed k is ~10x the cache magnitude, so its attention weight
# dominates the output and fp8 error there is not affordable. The x24 cache
# scale (and 1/sqrt(D)) folds into the softmax exp() input scale; the
# new-token score column is accumulated separately in bf16 and overwrites
# the fp8 matmul's (stale) CUR_POS column in PSUM.
SK = 24.0     # K cache fp8 scale
SW = 128.0    # wq/wk fp8 scale (absorbed by RMSNorm; 1/SW refolds into rope tables)
INVSD = 1.0 / np.sqrt(np.float32(128.0))
INVS = float(INVSD / SK)             # exp() input scale

N_CORES = 8
B = 32          # batch
T = 2048        # kv cache length (CUR_POS+1)
D = 128         # head dim
HQ = 4          # query heads per core
NQ = HQ * D     # 512
HID = 4096
KC = HID // D   # 32 contraction chunks of 128
EPS = 1e-6
CUR_POS = T - 1
NCHUNK = T // 128  # 16


def build_nc():
    nc = bacc.Bacc(
        "TRN2", target_bir_lowering=False, debug=False, num_devices=N_CORES
    )
    d = {}
    # weight/cache layouts are pre-swizzled on host to match the SBUF tiles
    # exactly, so every DMA is flat with large contiguous runs per partition
    for name, shape, dt in [
        ("xt", [D, KC * B], BF16),         # xt[p, 32c+b] = x[b, 128c+p]
        ("wqt", [8, 128, 2048], FP8),      # [g][p][(c n)] of wq-shard^T
        ("wkt", [2, 128, 2048], FP8),      # [half][p][(c n)] of wk-shard^T
        ("wvt", [2, 128, 2048], BF16),
        ("wot", [4, 128, HID], BF16),      # [h][d][o]: wo[o, 512i+128h+d]
        ("kt", [B // 2, D, 2 * T], FP8),  # [u][d][(j t)]: K^T, 2 batches/tile
        ("v", [B // 2, 128, 2 * T], FP8),  # [u][p][(j c e)]: V x24, 2 batches/tile
        ("cosq", [B, NQ], F32),            # rope cos for q, w&scale folded, x4
        ("sinq", [B, NQ], F32),            # rope sin (signed+permuted w), x4
        ("cosk", [B, D], F32),
        ("sink", [B, D], F32),
    ]:
        d[name] = nc.dram_tensor(name, shape, dt, kind="ExternalInput").ap()
    out_d = nc.dram_tensor("out", [B, HID], BF16, kind="ExternalOutput").ap()

    with tile.TileContext(nc) as tc:
        _build(tc, nc, d, out_d)
    nc.compile()
    return nc


def _build(tc, nc, d, out_d):
    with (
        tc.tile_pool(name="const", bufs=1) as const_pool,
        tc.tile_pool(name="small", bufs=1) as small,
        tc.tile_pool(name="big", bufs=1) as big,
        tc.tile_pool(name="kt_pool", bufs=16) as kt_pool,
        tc.tile_pool(name="wo_pool", bufs=4) as wo_pool,
        tc.tile_pool(name="dram", bufs=1, space="DRAM") as dram_pool,
    ):
        eye = const_pool.tile([128, 128], F32)
        make_identity(nc, eye[:])

        # touch the Exp table now so the ~1.5us ACT_TABLE_LOAD happens
        # during the DMA ramp instead of on the softmax critical path
        # (dependency-free: the ACT queue must not wait before issuing the
        # weight-stream DMAs queued behind this)
        warm = small.tile([1, 1], F32, tag="warm")
        nc.vector.memset(warm[:], 0.0)
        nc.scalar.activation(warm[:], warm[:], AF.Exp)

        # qpad zero-fill first: no deps, runs at t=0 off the critical path
        qpad = big.tile([128, B * 128], BF16, tag="qpad")
        nc.vector.memset(qpad[:], 0.0)

        kT_sb = small.tile([D, B], BF16)
        v_sb = small.tile([B, D], BF16)
        v_row0 = small.tile([1, B * D], BF16)

        wo_sb = []

        def fetch_wo():
            h = len(wo_sb)
            w = wo_pool.tile([128, HID], BF16, tag="wo", name=f"wo{h}")
            nc.sync.dma_start(w[:], d["wot"][h])
            wo_sb.append(w)

        def warm_pe_on(src_ap, scratch_ps):
            # PE-warmth keeper: throwaway matmul gated on src_ap's
            # producer; keeps the HAM activity window non-idle through
            # DMA-paced lulls so the next real burst runs at 2.4 GHz.
            nc.tensor.matmul(
                scratch_ps, src_ap, src_ap,
                start=True, stop=True, skip_group_check=True,
            )

        # ---------------- Phase A: QKV projection ----------------
        with (
            tc.tile_pool(name="pb", bufs=1) as pb,
            tc.tile_pool(name="wq_pool", bufs=8) as wq_pool,
            tc.tile_pool(name="wkv_pool", bufs=1) as wkv_pool,
            tc.tile_pool(name="ps_qkv", bufs=1, space="PSUM") as ps_qkv,
        ):
            # EVERYTHING rides the single sync ring in exact consumption
            # order: x, wq, wk, wv, rope, kt0..15, v0..15, wo0..3, out.
            # Phase-A inputs at the head get full HBM bandwidth (~12us)
            # instead of dribbling in behind the kt stream.
            x_sb = pb.tile([D, KC * B], BF16)
            nc.sync.dma_start(x_sb[:], d["xt"][:])

            wk_sb = wkv_pool.tile([128, HID], FP8, tag="wk")
            wv_sb = wkv_pool.tile([128, HID], BF16, tag="wv")
            wq_tiles = []
            for g in range(8):
                w = wq_pool.tile([128, 2048], FP8, tag="wq", name=f"wq{g}")
                nc.sync.dma_start(w[:], d["wqt"][g])
                wq_tiles.append(w)
            nc.sync.dma_start(wk_sb[:, 0:2048], d["wkt"][0])
            nc.sync.dma_start(wk_sb[:, 2048:4096], d["wkt"][1])
            nc.sync.dma_start(wv_sb[:, 0:2048], d["wvt"][0])
            nc.sync.dma_start(wv_sb[:, 2048:4096], d["wvt"][1])

            q_ps = ps_qkv.tile([B, NQ], F32, tag="q")
            k_ps = ps_qkv.tile([B, D], F32, tag="k")
            v_ps = ps_qkv.tile([B, D], F32, tag="v")

            # separate loops: PE queue is FIFO, so k/v matmuls (whose weights
            # arrive after wq) must not block the q stream
            for c in range(KC):
                nc.tensor.matmul(
                    q_ps[:], x_sb[:, ts(c, B)],
                    wq_tiles[c // 4][:, ts(c % 4, NQ)],
                    start=(c == 0), stop=(c == KC - 1),
                )

            # q RMSNorm stats (DVE/ACT run these while PE does k/v matmuls)
            q_sb = pb.tile([B, NQ], F32)
            nc.scalar.copy(q_sb[:], q_ps[:])
            qsq = pb.tile([B, NQ], F32)
            nc.scalar.square(qsq[:], q_ps[:])

            for c in range(KC):
                nc.tensor.matmul(
                    k_ps[:], x_sb[:, ts(c, B)], wk_sb[:, ts(c, D)],
                    start=(c == 0), stop=(c == KC - 1),
                )
            for c in range(KC):
                nc.tensor.matmul(
                    v_ps[:], x_sb[:, ts(c, B)], wv_sb[:, ts(c, D)],
                    start=(c == 0), stop=(c == KC - 1),
                )

            k_sb = pb.tile([B, D], F32)
            nc.scalar.copy(k_sb[:], k_ps[:])
            ksq = pb.tile([B, D], F32)
            nc.scalar.square(ksq[:], k_ps[:])
            # v_sb = v_new * SK (bf16): the newtok AV matmul's pT row
            # carries probs/SK (the fp8 V-cache descale is folded into the
            # softmax diag), so the exact-bf16 newtok V pre-multiplies SK.
            nc.vector.tensor_scalar_mul(v_sb[:], v_ps[:], SK)
            # v_row0[0, b*128+e] = v_sb[b, e]: flatten all batches' new-token
            # v onto partition 0 (where the K=1 newtok AV matmuls can reach
            # it) via a DRAM bounce -- a direct cross-partition SBUF->SBUF
            # reshape is not expressible as a safe AP pair.
            vb_d = dram_pool.tile([B, D], BF16, tag="vbounce")
            nc.gpsimd.dma_start(vb_d[:], v_sb[:])
            nc.gpsimd.dma_start(
                v_row0[:].rearrange("p (b e) -> p b e", b=B),
                vb_d[:].rearrange("(q b) e -> q b e", q=1),
            )

            # ---------------- Phase B: RMSNorm + RoPE ----------------
            ssq_q = pb.tile([B, HQ], F32)
            nc.vector.reduce_sum(
                ssq_q[:], qsq[:].rearrange("p (h e) -> p h e", e=D), axis=AX.X
            )
            ssq_k = pb.tile([B, 1], F32)
            nc.vector.reduce_sum(ssq_k[:], ksq[:], axis=AX.X)

            # rstd = sqrt(1 / (ssq/D + eps))
            rstd_q = pb.tile([B, HQ], F32)
            nc.vector.tensor_scalar(
                rstd_q[:], ssq_q[:], 1.0 / (D * SW * SW), EPS,
                op0=ALU.mult, op1=ALU.add,
            )
            nc.vector.reciprocal(rstd_q[:], rstd_q[:])
            nc.scalar.sqrt(rstd_q[:], rstd_q[:])
            rstd_k = pb.tile([B, 1], F32)
            nc.vector.tensor_scalar(
                rstd_k[:], ssq_k[:], 1.0 / (D * SW * SW), EPS,
                op0=ALU.mult, op1=ALU.add,
            )
            nc.vector.reciprocal(rstd_k[:], rstd_k[:])
            nc.scalar.sqrt(rstd_k[:], rstd_k[:])

            qn = pb.tile([B, NQ], F32)
            for h in range(HQ):
                nc.vector.tensor_scalar_mul(
                    qn[:, ts(h, D)], q_sb[:, ts(h, D)], rstd_q[:, h : h + 1]
                )
            kn = pb.tile([B, D], F32)
            nc.vector.tensor_scalar_mul(kn[:], k_sb[:], rstd_k[:, 0:1])

            # RoPE: out = x*cos + perm(x)*sin_signed (w, 1/sqrt(D) host-folded)
            cq = pb.tile([B, NQ], F32)
            nc.sync.dma_start(cq[:], d["cosq"][:])
            sq = pb.tile([B, NQ], F32)
            nc.sync.dma_start(sq[:], d["sinq"][:])
            ck = pb.tile([B, D], F32)
            nc.sync.dma_start(ck[:], d["cosk"][:])
            sk = pb.tile([B, D], F32)
            nc.sync.dma_start(sk[:], d["sink"][:])

            def rope(dst, xin, cos_t, sin_t, nh):
                tcos = pb.tile([B, nh * D], F32, tag=f"tcos{nh}")
                nc.vector.tensor_mul(tcos[:], xin[:], cos_t[:])
                trot = pb.tile([B, nh * D], F32, tag=f"trot{nh}")
                x_r = xin[:].rearrange("p (h e) -> p h e", e=D)
                s_r = sin_t[:].rearrange("p (h e) -> p h e", e=D)
                t_r = trot[:].rearrange("p (h e) -> p h e", e=D)
                nc.vector.tensor_mul(
                    t_r[:, :, 0 : D // 2], x_r[:, :, D // 2 : D],
                    s_r[:, :, 0 : D // 2],
                )
                nc.vector.tensor_mul(
                    t_r[:, :, D // 2 : D], x_r[:, :, 0 : D // 2],
                    s_r[:, :, D // 2 : D],
                )
                nc.vector.tensor_add(dst[:], tcos[:], trot[:])

            q_fin = pb.tile([B, NQ], F32)
            rope(q_fin, qn, cq, sq, HQ)
            k_fin = pb.tile([B, D], F32)
            rope(k_fin, kn, ck, sk, 1)

            # ---------------- Q^T / K^T assembly ----------------
            # Q^T columns land directly in the zero-padded per-batch lhsT
            # tiles: tile b holds Q^T cols of batch b at columns 4b..4b+4
            # (zeros elsewhere), so the psum-accumulated scores fill all 128
            # (b,h) rows with no junk. qpad col for (b,h) = 132*b + h.
            # The psum->sbuf copy also casts f32 -> bf16.
            for h in range(HQ):
                tp = ps_qkv.tile([128, 128], F32, tag="tp")
                nc.tensor.transpose(
                    tp[:, 0:B], q_fin[:, ts(h, D)], eye[0:B, 0:B]
                )
                nc.vector.tensor_copy(qpad[:, h : B * 128 : 132], tp[:, 0:B])
            tp = ps_qkv.tile([128, 128], F32, tag="tp")
            nc.tensor.transpose(tp[:, 0:B], k_fin[:], eye[0:B, 0:B])
            nc.vector.tensor_copy(kT_sb[:], tp[:, 0:B])

        # ---------------- Pass 1: scores + softmax ----------------
        attn = big.tile([128, T], BF16, tag="attn")
        sums = small.tile([128, 1], F32)

        v_tiles = {}

        def fetch_v(u):
            # all big streams ride the sync ring in exact consumption
            # order. V rides fp8 (x24); the host rolled chunk 15 so the
            # new-token slot t=2047 sits at partition 0 AND zeroed it, so
            # no device-side patch DMA is needed at all -- the new token's
            # exact-bf16 contribution comes from a separate K=1 matmul.
            vtile = v_pool.tile([128, 2 * T], FP8, tag="v", name=f"v{u}")
            nc.sync.dma_start(vtile[:], d["v"][u])
            v_tiles[u] = vtile

        # v pool opens AFTER phase A closed so the allocator reuses the
        # pb/wq/wkv SBUF zones (their readers finish early, so the
        # released-zone deps are free); 12 bufs so the sync ring never
        # idles long on a pool-WAR gate. wo stays static in the outer
        # scope: a zone-reuse dep there held its DMAs back 28us.
        with (
            tc.tile_pool(name="v_pool", bufs=16) as v_pool,
        ):
            with (
                tc.tile_pool(name="ps_sc", bufs=1, space="PSUM") as ps_sc,
                tc.tile_pool(name="ps_av", bufs=2, space="PSUM") as ps_av,
                tc.tile_pool(name="ps_tp", bufs=2, space="PSUM") as ps_tp,
            ):
                sc = [
                    ps_sc.tile([128, 512], F32, tag=f"sc{c}", name=f"sc{c}")
                    for c in range(4)
                ]
                # ALL 8 AV accumulators live as column slices of two PSUM
                # banks -- zero buffer reuse, so no WAR micro-gap between
                # groups ever resets the PE's HAM warm-up window (the gaps
                # kept the whole AV phase clock-gated at 1.2 GHz).
                av_bigs = [
                    ps_av.tile([128, 512], F32, tag="av", name=f"avbig{t}")
                    for t in range(2)
                ]

                # the whole sync-ring stream queues up front in exact
                # consumption order: kt0..15, v0..15 (+newtok column
                # patches), wo0..3, out DMAs. Pool WAR semaphores pace the
                # issue against consumption; nothing else rides sync.
                kt_tiles = {}

                def fetch_kt(u):
                    ktile = kt_pool.tile(
                        [D, 2 * T], FP8, tag="kt", name=f"kt{u}"
                    )
                    nc.sync.dma_start(ktile[:], d["kt"][u])
                    kt_tiles[u] = ktile

                # kt pairs (u, u+8) arrive adjacently for score_pair's
                # two concurrent col-tiled streams, with a v pair woven
                # between each: all of V is on-chip by the time softmax
                # ends, so the AV phase runs DENSE (no v-paced idle gaps
                # that kept re-throttling the PE clock to 1.2 GHz) while
                # wo's arrival -- the real o_proj gate -- is unchanged.
                # bufs=16 on both pools = no WAR gates anywhere.
                for u in range(8):
                    fetch_kt(u)
                    fetch_kt(u + 8)
                    fetch_v(2 * u)
                    fetch_v(2 * u + 1)
                for _ in range(4):
                    fetch_wo()

                psum = [
                    small.tile([128, 1], F32, tag=f"psums{c}", name=f"psum{c}")
                    for c in range(4)
                ]
                rs = small.tile([128, 1], F32)
                diag_rs = small.tile([128, 128], BF16)
                # attn^T chunks: pT[t_chunk, (b,h)] for the AV contraction
                pT = big.tile([128, T], BF16, tag="pT")  # free = (c, bh)

                # oT holds attn-out transposed HEAD-major: col 32h + b
                # (contraction chunk h of o_proj = contiguous cols 32h..)
                oT_sb = small.tile([D, B * HQ], BF16)

                pending = []

                def flush_group():
                    av_sb4, fg = pending.pop(0)
                    tp4 = ps_tp.tile(
                        [128, 128], F32, tag="tp", name=f"tpo{fg}"
                    )
                    nc.tensor.transpose(tp4[:], av_sb4[:], eye[:])
                    # tp4 col 32j+h holds (batch 4*fg+j, head h); scatter the
                    # 4x4 used cols into oT[d, 32h + 4fg + j] (cast f32->bf16)
                    # -- one copy per head so every oT write is a contiguous
                    # column slice (plain dst APs for the dep tracker)
                    tp4_r = tp4[:].rearrange("p (j z) -> p j z", j=4)
                    for h in range(HQ):
                        nc.vector.tensor_copy(
                            oT_sb[:, 32 * h + 4 * fg : 32 * h + 4 * fg + 4],
                            tp4_r[:, :, h],
                        )

                def score_pair(u):
                    """Scores for kt tiles u (half 0) and u+8 (half 1) as
                    TWO CONCURRENT col-tiled PE streams: half 0's 64-col
                    qpad lhsT loads array cols 0-63 (tile_position (0,0)),
                    half 1's loads cols 64-127 ((0,64)); each stream pulls
                    its own kt rhs through its own XBUS pair and lands in
                    its own partition range of the shared sc banks. Halves
                    that used to serialize (27.6us) now overlap (~14us)."""
                    ka = kt_tiles.pop(u)
                    kb = kt_tiles.pop(u + 8)
                    for j in range(2):
                        ba = 2 * u + j
                        bb = 2 * (u + 8) + j
                        for c in range(4):
                            nc.tensor.matmul(
                                sc[c][0:64, :],
                                qpad[:, 128 * ba : 128 * ba + 64],
                                ka[:, j * T + 512 * c : j * T + 512 * (c + 1)],
                                start=(ba == 0), stop=(ba == 15),
                                tile_position=(0, 0),
                                skip_group_check=True,
                            )
                            nc.tensor.matmul(
                                sc[c][64:128, :],
                                qpad[:, 128 * bb + 64 : 128 * bb + 128],
                                kb[:, j * T + 512 * c : j * T + 512 * (c + 1)],
                                start=(bb == 16), stop=(bb == 31),
                                tile_position=(0, 64),
                                skip_group_check=True,
                            )

                def newtok_all():
                    # new-token score column in bf16, both halves as
                    # concurrent col-tiled pairs. The host rolled chunk 15's
                    # stream order to [2047, 1920..2046] and zeroed the
                    # stale slot, so t=2047 lives at stream col 1920 =
                    # sc[3] col 384; start=True overwrites the fp8 matmul's
                    # zero contribution there.
                    for b in range(16):
                        bb = b + 16
                        nc.tensor.matmul(
                            sc[3][0:64, 384:385],
                            qpad[:, 128 * b : 128 * b + 64],
                            kT_sb[:, b : b + 1],
                            start=(b == 0), stop=(b == 15),
                            tile_position=(0, 0),
                            skip_group_check=True,
                        )
                        nc.tensor.matmul(
                            sc[3][64:128, 384:385],
                            qpad[:, 128 * bb + 64 : 128 * bb + 128],
                            kT_sb[:, bb : bb + 1],
                            start=(bb == 16), stop=(bb == 31),
                            tile_position=(0, 64),
                            skip_group_check=True,
                        )

                def softmax_all():
                    # merged full-128-row softmax (both halves finish
                    # together now): ACT/DVE cost per op is free-dim bound,
                    # so [128,512] ops cost the same as [64,512] -- half
                    # the chain of the old per-half version. No max
                    # subtraction: |score*INVS| <= ~13 by Cauchy-Schwarz,
                    # far below f32 exp overflow (88).
                    for c in range(4):
                        nc.scalar.activation(
                            attn[:, ts(c, 512)], sc[c][:, :], AF.Exp,
                            scale=INVS, accum_out=psum[c][:],
                        )
                        # PE-warmth keeper: a throwaway matmul gated on
                        # this exp's output (writing into the now-dead sc
                        # bank) fires every ~0.7us through the softmax
                        # chain, so the HAM activity monitor never sees an
                        # idle MID window here -- one re-throttle at this
                        # point put the ENTIRE AV phase at 1.2 GHz
                        # (measured 34us cold, ~15us of loss).
                        nc.tensor.matmul(
                            sc[c][0:64, 0:64],
                            attn[:, c * 512 : c * 512 + 64],
                            attn[:, c * 512 : c * 512 + 64],
                            start=True, stop=True, skip_group_check=True,
                        )
                    nc.vector.tensor_add(psum[0][:], psum[0][:], psum[1][:])
                    nc.vector.tensor_add(psum[2][:], psum[2][:], psum[3][:])
                    nc.vector.tensor_add(sums[:], psum[0][:], psum[2][:])
                    # x SK: folds the fp8 V-cache descale (V stored x24)
                    # into the softmax normalization: pT = probs/SK
                    nc.vector.tensor_scalar_mul(sums[:], sums[:], SK)
                    nc.vector.reciprocal(rs[:], sums[:])
                    # diag(1/sum): folds normalization into the pT matmuls
                    nc.vector.tensor_scalar_mul(
                        diag_rs[:], eye[:], rs[:, 0:1]
                    )
                    nc.tensor.matmul(
                        sc[0][0:64, 0:64], diag_rs[:, 0:64],
                        diag_rs[:, 0:64],
                        start=True, stop=True, skip_group_check=True,
                    )
                    for c in range(NCHUNK):
                        tp = ps_tp.tile([128, 128], F32, tag="tp")
                        nc.tensor.matmul(
                            tp[:], attn[:, ts(c, 128)], diag_rs[:]
                        )
                        nc.vector.tensor_copy(pT[:, ts(c, 128)], tp[:])

                def av_group(g):
                    # Col-tiled AV: 4 batches packed per PSUM tile at
                    # partition bases 0/32/64/96 via tile_position; the 4
                    # streams run concurrently on the PE
                    vt0 = v_tiles.pop(2 * g)
                    vt1 = v_tiles.pop(2 * g + 1)
                    av4 = av_bigs[g // 4][:, ts(g % 4, D)]
                    for c in range(NCHUNK):
                        for j in range(4):
                            b = 4 * g + j
                            vt = vt0 if j < 2 else vt1
                            jj = j % 2
                            nc.tensor.matmul(
                                av4[32 * j : 32 * j + HQ, :],
                                pT[:, c * 128 + HQ * b : c * 128 + HQ * b + HQ],
                                vt[:, jj * T + c * D : jj * T + (c + 1) * D],
                                start=(c == 0), stop=False,
                                tile_position=(0, 32 * j),
                                skip_group_check=True,
                            )
                    # exact-bf16 new-token term: K=1 matmul at partition 0
                    # (pT chunk-15 row 0 = t 2047 prob/SK; v_row0 = v_new*SK)
                    for j in range(4):
                        b = 4 * g + j
                        nc.tensor.matmul(
                            av4[32 * j : 32 * j + HQ, :],
                            pT[0:1, 15 * 128 + HQ * b : 15 * 128 + HQ * b + HQ],
                            v_row0[0:1, b * D : (b + 1) * D],
                            start=False, stop=True,
                            tile_position=(0, 32 * j),
                            skip_group_check=True,
                        )
                    av_sb4 = small.tile(
                        [128, D], F32, tag="av_sb", bufs=3, name=f"avs{g}"
                    )
                    nc.scalar.copy(av_sb4[:], av4[:, :])
                    pending.append((av_sb4, g))
                    # flush one group late: the PE transpose of group g-1
                    # runs behind group g's AV matmuls, so the copy's
                    # cross-engine round-trip never idles the PE
                    if len(pending) > 1:
                        flush_group()

                for u in range(8):
                    score_pair(u)
                newtok_all()
                softmax_all()
                for g in range(8):
                    av_group(g)
                while pending:
                    flush_group()

            # ---------------- o_proj: partial column shard ----------------
            # out_partial[32, 4096] = attn_local[32, 512] @ wo_col_shard.T,
            # accumulated over 4 head chunks (contiguous oT cols as lhsT).
            # 4x col-tiled: two [128,512] PSUM tiles each pack 4 output
            # n-banks at partition bases 0/32/64/96, so 4 n-banks' streams
            # run concurrently (~1.8us instead of 7). h-OUTER so the MM
            # stream starts as soon as wo[0] lands; during the final h
            # sweep each slice's copy + 64KB out-DMA pipeline behind the
            # next slice's matmul.
            with tc.tile_pool(name="ps_o", bufs=2, space="PSUM") as ps_o:
                o_big = [
                    ps_o.tile([128, 512], F32, tag="o", name=f"obig{p}")
                    for p in range(2)
                ]
                for h in range(HQ):
                    for p in range(2):
                        for q in range(4):
                            n = 4 * p + q
                            nc.tensor.matmul(
                                o_big[p][32 * q : 32 * q + B, :],
                                oT_sb[:, 32 * h : 32 * h + 32],
                                wo_sb[h][:, ts(n, 512)],
                                start=(h == 0), stop=(h == HQ - 1),
                                tile_position=(0, 32 * q),
                                skip_group_check=True,
                            )
                            if h == HQ - 1:
                                # bf16 partials (host sums them in f32:
                                # each partial is ~1/8 of the output, so
                                # the rounding adds ~0.07% -- negligible);
                                # out DMAs alternate sync/gpsimd so the 8
                                # issues don't serialize on one ring at
                                # the very tail.
                                o_sb = small.tile(
                                    [B, 512], BF16, tag="osb", bufs=4,
                                    name=f"osb{n}",
                                )
                                sl = o_big[p][32 * q : 32 * q + B, :]
                                if n % 2 == 0:
                                    nc.scalar.copy(o_sb[:], sl)
                                else:
                                    nc.vector.tensor_copy(o_sb[:], sl)
                                eng = nc.sync if n % 2 == 0 else nc.gpsimd
                                eng.dma_start(
                                    out_d[:, ts(n, 512)], o_sb[:]
                                )


def _install_ntff_hook():
    """The agent image's antenv lacks axon_hooks; register an equivalent that
    drives NTFF profiling via ctypes into the injected libaxon_pjrt.so, so
    run_bass_kernel_spmd(trace=True) can capture HW exec times."""
    import types, ctypes, contextlib

    try:
        from antenv.axon_hooks import get_axon_ntff_profile_hook  # noqa: F401
        return  # real one exists
    except ImportError:
        pass
    so_path = "/opt/axon/libaxon_pjrt.so"
    try:
        lib = ctypes.CDLL(so_path)
        if not hasattr(lib, "axon_start_nrt_profile"):
            return
    except OSError:
        return
    lib.axon_start_nrt_profile.argtypes = [
        ctypes.POINTER(ctypes.c_int64), ctypes.c_size_t,
    ]
    lib.axon_start_nrt_profile.restype = ctypes.c_int64
    lib.axon_stop_nrt_profile.argtypes = [ctypes.c_char_p]
    lib.axon_stop_nrt_profile.restype = ctypes.c_int64

    @contextlib.contextmanager
    def _hook(output_dir, device_ids):
        import jax

        jax.devices()
        if device_ids:
            ids = (ctypes.c_int64 * len(device_ids))(*device_ids)
            rc = lib.axon_start_nrt_profile(ids, len(device_ids))
        else:
            rc = lib.axon_start_nrt_profile(None, 0)
        if rc != 0:
            raise RuntimeError(f"axon_start_nrt_profile rc={rc}")
        try:
            yield
        finally:
            n = lib.axon_stop_nrt_profile(str(output_dir).encode())
            print(f"ntff profile: {n} file(s) written to {output_dir}")

    mod = types.ModuleType("antenv.axon_hooks")
    mod.get_axon_ntff_profile_hook = lambda: _hook
    mod.set_axon_ntff_profile_hook = lambda h: None
    sys.modules["antenv.axon_hooks"] = mod


_NC_CACHE = None


def _get_nc():
    global _NC_CACHE
    if _NC_CACHE is None:
        _NC_CACHE = build_nc()
    return _NC_CACHE


def _prep_inputs(x, wq, wk, wv, wo, q_norm_w, k_norm_w, cos, sin,
                 k_cache, v_cache, position_ids):
    x = np.asarray(x, np.float32).reshape(B, HID)
    pids = np.asarray(position_ids).reshape(B).astype(np.int64)
    cos_g = np.asarray(cos, np.float32)[pids]  # [B, D]
    sin_g = np.asarray(sin, np.float32)[pids]
    qw = np.asarray(q_norm_w, np.float32)
    kw = np.asarray(k_norm_w, np.float32)
    perm = (np.arange(D) + D // 2) % D
    sgn = np.where(np.arange(D) < D // 2, -1.0, 1.0).astype(np.float32)
    # 1/sqrt(D) folds into the exp() input scale (INVS); 1/SW undoes the
    # wq fp8 scale after the (scale-invariant) RMSNorm
    cosq1 = cos_g * qw[None, :] / SW
    sinq1 = sgn[None, :] * sin_g * qw[perm][None, :] / SW
    cosq = np.ascontiguousarray(np.tile(cosq1, (1, HQ)))
    sinq = np.ascontiguousarray(np.tile(sinq1, (1, HQ)))
    # k_fin lands at SK x true scale to match the fp8 cache column scale;
    # 1/SW undoes the wk fp8 scale after the scale-invariant RMSNorm
    fk = SK / SW
    cosk = np.ascontiguousarray(cos_g * kw[None, :] * fk)
    sink = np.ascontiguousarray(sgn[None, :] * sin_g * kw[perm][None, :] * fk)

    # xt[p, 32c+b] = x[b, 128c+p]
    xt = np.ascontiguousarray(
        x.T.reshape(KC, D, B).transpose(1, 0, 2).reshape(D, KC * B)
    ).astype(NP_BF16)

    wq = np.asarray(wq, np.float32)
    wk = np.asarray(wk, np.float32)
    wv = np.asarray(wv, np.float32)
    wo = np.asarray(wo, np.float32)
    kc_np = np.asarray(k_cache, np.float32)
    vc_np = np.asarray(v_cache, np.float32)

    in_maps = []
    for i in range(N_CORES):
        m = dict(xt=xt, cosq=cosq, sinq=sinq, cosk=cosk, sink=sink)
        # [g][p][(c n)]: group g holds contraction chunks 4g..4g+4
        wqt = wq[i * NQ : (i + 1) * NQ, :].T.reshape(8, 4, 128, NQ)
        m["wqt"] = (np.ascontiguousarray(wqt.transpose(0, 2, 1, 3)).reshape(
            8, 128, 2048
        ) * SW).astype(NP_FP8)
        wkt = wk[i * D : (i + 1) * D, :].T.reshape(2, 16, 128, D)
        m["wkt"] = (np.ascontiguousarray(wkt.transpose(0, 2, 1, 3)).reshape(
            2, 128, 2048
        ) * SW).astype(NP_FP8)
        wvt = wv[i * D : (i + 1) * D, :].T.reshape(2, 16, 128, D)
        m["wvt"] = np.ascontiguousarray(wvt.transpose(0, 2, 1, 3)).reshape(
            2, 128, 2048
        ).astype(NP_BF16)
        # wot[h][d][o] = wo[o, 512i + 128h + d] (column shard, pre-T)
        wot = wo[:, i * NQ : (i + 1) * NQ].reshape(HID, HQ, D)
        m["wot"] = np.ascontiguousarray(wot.transpose(1, 2, 0)).astype(
            NP_BF16
        )
        # kt[u][d][(j t)] = K^T; v[u][p][(j c e)] with stream slot = 128c+p.
        # Chunk 15's stream order is rolled to [2047, 1920..2046] so the
        # new token t=2047 sits at partition 0 of the V tile (and pT row 0)
        # where the K=1 exact-bf16 newtok matmul can address it; the rolled
        # slot's stale cache values are ZEROED (k and v), so the fp8
        # score/AV matmuls contribute nothing there and no device-side
        # patch DMA is needed.
        perm_t = np.concatenate(
            [np.arange(1920), [2047], np.arange(1920, 2047)]
        )
        kti = kc_np[0, :, :, i, :][:, perm_t, :]   # [B, T, D] (copy)
        kti[:, 1920, :] = 0.0
        kti = kti.transpose(0, 2, 1).reshape(B // 2, 2, D, T)
        m["kt"] = (np.ascontiguousarray(kti.transpose(0, 2, 1, 3)).reshape(
            B // 2, D, 2 * T
        ) * SK).astype(NP_FP8)
        vi = vc_np[0, :, :, i, :][:, perm_t, :]    # [B, T, D] (copy)
        vi[:, 1920, :] = 0.0
        vi = vi.reshape(B // 2, 2, NCHUNK, 128, D)
        m["v"] = (np.ascontiguousarray(vi.transpose(0, 3, 1, 2, 4)).reshape(
            B // 2, 128, 2 * T
        ) * SK).astype(NP_FP8)
        in_maps.append(m)
    return in_maps


def kernel(x, wq, wk, wv, wo, q_norm_w, k_norm_w, cos, sin,
           k_cache, v_cache, position_ids, _trace=False, _trace_cores=None):
    nc = _get_nc()
    if _trace:
        _install_ntff_hook()
    in_maps = _prep_inputs(x, wq, wk, wv, wo, q_norm_w, k_norm_w, cos, sin,
                           k_cache, v_cache, position_ids)
    res = run_bass_kernel_spmd(
        nc, in_maps, core_ids=list(range(N_CORES)),
        trace=_trace, trace_cores=_trace_cores,
    )
    out = np.sum(
        [np.asarray(res.results[i]["out"], np.float32) for i in range(N_CORES)],
        axis=0, dtype=np.float32,
    ).reshape(B, 1, HID)
    if _trace:
        return out, res
    return out



# revision 55
# speedup vs baseline: 1.3895x; 1.3895x over previous
"""GQA attention decode step (B=32, S=1, H=32, KVH=8, D=128, HID=4096, T=2048)
on 8 Trainium2 NeuronCores, tensor-parallel over heads.

Sharding: core i owns query heads 4i..4i+3 and kv head i. Each core: QKV
proj (x @ w shards) -> per-head RMSNorm + RoPE -> attention over its
kv-head's 2048-entry cache (all 32 batches) -> Megatron-style PARTIAL
o_proj against the 512-wide COLUMN shard of wo. No collective: the host
sums the 8 [32,4096] f32 partials (that is the unshard step).

Precision: fp8 e3m4 for wq/wk (x128, absorbed by the scale-invariant q/k
RMSNorms) AND both caches (x24; K's scale folds into the softmax exp()
input scale, V's into the normalization diag). The NEW token's k/v stay
exact bf16: its score column accumulates over the fp8 matmul's zeroed
column (host zeroes the stale slot), and its AV term is a separate K=1
matmul at partition 0 against v_row0 (host ROLLS chunk 15's stream order
to [2047, 1920..2046] so t=2047 sits at partition 0 / pT row 0). Softmax
runs WITHOUT max subtraction (|score*INVS| <= ~13 << 88 = f32 exp
overflow). ~25.8MB HBM/core vs 89MB for the fp32 baseline.

Schedule: ONE sync-ring DMA stream in exact consumption order (x, wq, wk,
wv, rope, kt pairs, v, wo, out) -- order within a ring is the only
reliable arrival-order control, and pool WAR gates are sized away
(kt/v bufs=16). The two score halves run as CONCURRENT col-tiled PE
streams (half 0 in array cols 0-63, half 1 in 64-127, each pulling its
own kt tile), softmax is one merged full-128-row pass (ACT/DVE ops are
free-dim bound), AV packs 4 batches per PSUM bank via tile_position,
flushes write attn-out head-major (oT[d, 32h+b] = contiguous o_proj
lhsT chunks), and o_proj is 4x col-tiled (two [128,512] PSUM tiles hold
all 8 output banks). Throwaway "warmth" matmuls gated on the softmax
exps keep the PE's HAM clock at 2.4GHz through the exp/DVE window.
"""

import sys

sys.path.insert(0, "/opt/trn_rl_repo")

import numpy as np
import ml_dtypes

import concourse.bass as bass
import concourse.tile as tile
from concourse import bacc, mybir
from concourse.bass import ts
from concourse.bass_utils import run_bass_kernel_spmd
from concourse.masks import make_identity

F32 = mybir.dt.float32
BF16 = mybir.dt.bfloat16
FP8 = mybir.dt.float8e3
AF = mybir.ActivationFunctionType
ALU = mybir.AluOpType
AX = mybir.AxisListType

NP_BF16 = ml_dtypes.bfloat16
NP_FP8 = ml_dtypes.float8_e3m4

# Only the K cache rides in fp8 (e3m4, x24 so values sit in the normal
# range [0.25, 15.5]); everything touching the NEW token's k/v stays bf16 --
# the fresh RMSNormed k is ~10x the cache magnitude, so its attention weight
# dominates the output and fp8 error there is not affordable. The x24 cache
# scale (and 1/sqrt(D)) folds into the softmax exp() input scale; the
# new-token score column is accumulated separately in bf16 and overwrites
# the fp8 matmul's (stale) CUR_POS column in PSUM.
SK = 24.0     # K cache fp8 scale
SW = 128.0    # wq/wk fp8 scale (absorbed by RMSNorm; 1/SW refolds into rope tables)
INVSD = 1.0 / np.sqrt(np.float32(128.0))
INVS = float(INVSD / SK)             # exp() input scale

N_CORES = 8
B = 32          # batch
T = 2048        # kv cache length (CUR_POS+1)
D = 128         # head dim
HQ = 4          # query heads per core
NQ = HQ * D     # 512
HID = 4096
KC = HID // D   # 32 contraction chunks of 128
EPS = 1e-6
CUR_POS = T - 1
NCHUNK = T // 128  # 16


def build_nc():
    nc = bacc.Bacc(
        "TRN2", target_bir_lowering=False, debug=False, num_devices=N_CORES
    )
    d = {}
    # weight/cache layouts are pre-swizzled on host to match the SBUF tiles
    # exactly, so every DMA is flat with large contiguous runs per partition
    for name, shape, dt in [
        ("xt", [D, KC * B], BF16),         # xt[p, 32c+b] = x[b, 128c+p]
        ("wqt", [8, 128, 2048], FP8),      # [g][p][(c n)] of wq-shard^T
        ("wkt", [2, 128, 2048], FP8),      # [half][p][(c n)] of wk-shard^T
        ("wvt", [2, 128, 2048], BF16),
        ("wot", [4, 128, HID], BF16),      # [h][d][o]: wo[o, 512i+128h+d]
        ("kt", [B // 2, D, 2 * T], FP8),  # [u][d][(j t)]: K^T, 2 batches/tile
        ("v", [B // 2, 128, 2 * T], FP8),  # [u][p][(j c e)]: V x24, 2 batches/tile
        ("cosq", [B, NQ], F32),            # rope cos for q, w&scale folded, x4
        ("sinq", [B, NQ], F32),            # rope sin (signed+permuted w), x4
        ("cosk", [B, D], F32),
        ("sink", [B, D], F32),
    ]:
        d[name] = nc.dram_tensor(name, shape, dt, kind="ExternalInput").ap()
    out_d = nc.dram_tensor("out", [B, HID], BF16, kind="ExternalOutput").ap()

    with tile.TileContext(nc) as tc:
        _build(tc, nc, d, out_d)
    nc.compile()
    return nc


def _build(tc, nc, d, out_d):
    with (
        tc.tile_pool(name="const", bufs=1) as const_pool,
        tc.tile_pool(name="small", bufs=1) as small,
        tc.tile_pool(name="big", bufs=1) as big,
        tc.tile_pool(name="kt_pool", bufs=16) as kt_pool,
        tc.tile_pool(name="wo_pool", bufs=4) as wo_pool,
        tc.tile_pool(name="dram", bufs=1, space="DRAM") as dram_pool,
    ):
        eye = const_pool.tile([128, 128], F32)
        make_identity(nc, eye[:])

        # touch the Exp table now so the ~1.5us ACT_TABLE_LOAD happens
        # during the DMA ramp instead of on the softmax critical path
        # (dependency-free: the ACT queue must not wait before issuing the
        # weight-stream DMAs queued behind this)
        warm = small.tile([1, 1], F32, tag="warm")
        nc.vector.memset(warm[:], 0.0)
        nc.scalar.activation(warm[:], warm[:], AF.Exp)

        # qpad zero-fill first: no deps, runs at t=0 off the critical path
        qpad = big.tile([128, B * 128], BF16, tag="qpad")
        nc.vector.memset(qpad[:], 0.0)

        kT_sb = small.tile([D, B], BF16)
        v_sb = small.tile([B, D], BF16)
        v_row0 = small.tile([1, B * D], BF16)

        wo_sb = []

        def fetch_wo():
            h = len(wo_sb)
            w = wo_pool.tile([128, HID], BF16, tag="wo", name=f"wo{h}")
            nc.sync.dma_start(w[:], d["wot"][h])
            wo_sb.append(w)

        def warm_pe_on(src_ap, scratch_ps):
            # PE-warmth keeper: throwaway matmul gated on src_ap's
            # producer; keeps the HAM activity window non-idle through
            # DMA-paced lulls so the next real burst runs at 2.4 GHz.
            nc.tensor.matmul(
                scratch_ps, src_ap, src_ap,
                start=True, stop=True, skip_group_check=True,
            )

        # ---------------- Phase A: QKV projection ----------------
        with (
            tc.tile_pool(name="pb", bufs=1) as pb,
            tc.tile_pool(name="wq_pool", bufs=8) as wq_pool,
            tc.tile_pool(name="wkv_pool", bufs=1) as wkv_pool,
            tc.tile_pool(name="ps_qkv", bufs=1, space="PSUM") as ps_qkv,
        ):
            # EVERYTHING rides the single sync ring in exact consumption
            # order: x, wq, wk, wv, rope, kt0..15, v0..15, wo0..3, out.
            # Phase-A inputs at the head get full HBM bandwidth (~12us)
            # instead of dribbling in behind the kt stream.
            x_sb = pb.tile([D, KC * B], BF16)
            nc.sync.dma_start(x_sb[:], d["xt"][:])

            wk_sb = wkv_pool.tile([128, HID], FP8, tag="wk")
            wv_sb = wkv_pool.tile([128, HID], BF16, tag="wv")
            wq_tiles = []
            for g in range(8):
                w = wq_pool.tile([128, 2048], FP8, tag="wq", name=f"wq{g}")
                nc.sync.dma_start(w[:], d["wqt"][g])
                wq_tiles.append(w)
            nc.sync.dma_start(wk_sb[:, 0:2048], d["wkt"][0])
            nc.sync.dma_start(wk_sb[:, 2048:4096], d["wkt"][1])
            nc.sync.dma_start(wv_sb[:, 0:2048], d["wvt"][0])
            nc.sync.dma_start(wv_sb[:, 2048:4096], d["wvt"][1])

            q_ps = ps_qkv.tile([B, NQ], F32, tag="q")
            k_ps = ps_qkv.tile([B, D], F32, tag="k")
            v_ps = ps_qkv.tile([B, D], F32, tag="v")

            # separate loops: PE queue is FIFO, so k/v matmuls (whose weights
            # arrive after wq) must not block the q stream
            for c in range(KC):
                nc.tensor.matmul(
                    q_ps[:], x_sb[:, ts(c, B)],
                    wq_tiles[c // 4][:, ts(c % 4, NQ)],
                    start=(c == 0), stop=(c == KC - 1),
                )

            # q RMSNorm stats (DVE/ACT run these while PE does k/v matmuls)
            q_sb = pb.tile([B, NQ], F32)
            nc.scalar.copy(q_sb[:], q_ps[:])
            qsq = pb.tile([B, NQ], F32)
            nc.scalar.square(qsq[:], q_ps[:])

            for c in range(KC):
                nc.tensor.matmul(
                    k_ps[:], x_sb[:, ts(c, B)], wk_sb[:, ts(c, D)],
                    start=(c == 0), stop=(c == KC - 1),
                )
            for c in range(KC):
                nc.tensor.matmul(
                    v_ps[:], x_sb[:, ts(c, B)], wv_sb[:, ts(c, D)],
                    start=(c == 0), stop=(c == KC - 1),
                )

            k_sb = pb.tile([B, D], F32)
            nc.scalar.copy(k_sb[:], k_ps[:])
            ksq = pb.tile([B, D], F32)
            nc.scalar.square(ksq[:], k_ps[:])
            # v_sb = v_new * SK (bf16): the newtok AV matmul's pT row
            # carries probs/SK (the fp8 V-cache descale is folded into the
            # softmax diag), so the exact-bf16 newtok V pre-multiplies SK.
            nc.vector.tensor_scalar_mul(v_sb[:], v_ps[:], SK)
            # v_row0[0, b*128+e] = v_sb[b, e]: flatten all batches' new-token
            # v onto partition 0 (where the K=1 newtok AV matmuls can reach
            # it) via a DRAM bounce -- a direct cross-partition SBUF->SBUF
            # reshape is not expressible as a safe AP pair.
            vb_d = dram_pool.tile([B, D], BF16, tag="vbounce")
            nc.gpsimd.dma_start(vb_d[:], v_sb[:])
            nc.gpsimd.dma_start(
                v_row0[:].rearrange("p (b e) -> p b e", b=B),
                vb_d[:].rearrange("(q b) e -> q b e", q=1),
            )

            # ---------------- Phase B: RMSNorm + RoPE ----------------
            ssq_q = pb.tile([B, HQ], F32)
            nc.vector.reduce_sum(
                ssq_q[:], qsq[:].rearrange("p (h e) -> p h e", e=D), axis=AX.X
            )
            ssq_k = pb.tile([B, 1], F32)
            nc.vector.reduce_sum(ssq_k[:], ksq[:], axis=AX.X)

            # rstd = sqrt(1 / (ssq/D + eps))
            rstd_q = pb.tile([B, HQ], F32)
            nc.vector.tensor_scalar(
                rstd_q[:], ssq_q[:], 1.0 / (D * SW * SW), EPS,
                op0=ALU.mult, op1=ALU.add,
            )
            nc.vector.reciprocal(rstd_q[:], rstd_q[:])
            nc.scalar.sqrt(rstd_q[:], rstd_q[:])
            rstd_k = pb.tile([B, 1], F32)
            nc.vector.tensor_scalar(
                rstd_k[:], ssq_k[:], 1.0 / (D * SW * SW), EPS,
                op0=ALU.mult, op1=ALU.add,
            )
            nc.vector.reciprocal(rstd_k[:], rstd_k[:])
            nc.scalar.sqrt(rstd_k[:], rstd_k[:])

            qn = pb.tile([B, NQ], F32)
            for h in range(HQ):
                nc.vector.tensor_scalar_mul(
                    qn[:, ts(h, D)], q_sb[:, ts(h, D)], rstd_q[:, h : h + 1]
                )
            kn = pb.tile([B, D], F32)
            nc.vector.tensor_scalar_mul(kn[:], k_sb[:], rstd_k[:, 0:1])

            # RoPE: out = x*cos + perm(x)*sin_signed (w, 1/sqrt(D) host-folded)
            cq = pb.tile([B, NQ], F32)
            nc.sync.dma_start(cq[:], d["cosq"][:])
            sq = pb.tile([B, NQ], F32)
            nc.sync.dma_start(sq[:], d["sinq"][:])
            ck = pb.tile([B, D], F32)
            nc.sync.dma_start(ck[:], d["cosk"][:])
            sk = pb.tile([B, D], F32)
            nc.sync.dma_start(sk[:], d["sink"][:])

            def rope(dst, xin, cos_t, sin_t, nh):
                tcos = pb.tile([B, nh * D], F32, tag=f"tcos{nh}")
                nc.vector.tensor_mul(tcos[:], xin[:], cos_t[:])
                trot = pb.tile([B, nh * D], F32, tag=f"trot{nh}")
                x_r = xin[:].rearrange("p (h e) -> p h e", e=D)
                s_r = sin_t[:].rearrange("p (h e) -> p h e", e=D)
                t_r = trot[:].rearrange("p (h e) -> p h e", e=D)
                nc.vector.tensor_mul(
                    t_r[:, :, 0 : D // 2], x_r[:, :, D // 2 : D],
                    s_r[:, :, 0 : D // 2],
                )
                nc.vector.tensor_mul(
                    t_r[:, :, D // 2 : D], x_r[:, :, 0 : D // 2],
                    s_r[:, :, D // 2 : D],
                )
                nc.vector.tensor_add(dst[:], tcos[:], trot[:])

            q_fin = pb.tile([B, NQ], F32)
            rope(q_fin, qn, cq, sq, HQ)
            k_fin = pb.tile([B, D], F32)
            rope(k_fin, kn, ck, sk, 1)

            # ---------------- Q^T / K^T assembly ----------------
            # Q^T columns land directly in the zero-padded per-batch lhsT
            # tiles: tile b holds Q^T cols of batch b at columns 4b..4b+4
            # (zeros elsewhere), so the psum-accumulated scores fill all 128
            # (b,h) rows with no junk. qpad col for (b,h) = 132*b + h.
            # The psum->sbuf copy also casts f32 -> bf16.
            for h in range(HQ):
                tp = ps_qkv.tile([128, 128], F32, tag="tp")
                nc.tensor.transpose(
                    tp[:, 0:B], q_fin[:, ts(h, D)], eye[0:B, 0:B]
                )
                nc.vector.tensor_copy(qpad[:, h : B * 128 : 132], tp[:, 0:B])
            tp = ps_qkv.tile([128, 128], F32, tag="tp")
            nc.tensor.transpose(tp[:, 0:B], k_fin[:], eye[0:B, 0:B])
            nc.vector.tensor_copy(kT_sb[:], tp[:, 0:B])

        # ---------------- Pass 1: scores + softmax ----------------
        attn = big.tile([128, T], BF16, tag="attn")
        sums = small.tile([128, 1], F32)

        v_tiles = {}

        def fetch_v(u):
            # all big streams ride the sync ring in exact consumption
            # order. V rides fp8 (x24); the host rolled chunk 15 so the
            # new-token slot t=2047 sits at partition 0 AND zeroed it, so
            # no device-side patch DMA is needed at all -- the new token's
            # exact-bf16 contribution comes from a separate K=1 matmul.
            vtile = v_pool.tile([128, 2 * T], FP8, tag="v", name=f"v{u}")
            nc.sync.dma_start(vtile[:], d["v"][u])
            v_tiles[u] = vtile

        # v pool opens AFTER phase A closed so the allocator reuses the
        # pb/wq/wkv SBUF zones (their readers finish early, so the
        # released-zone deps are free); 12 bufs so the sync ring never
        # idles long on a pool-WAR gate. wo stays static in the outer
        # scope: a zone-reuse dep there held its DMAs back 28us.
        with (
            tc.tile_pool(name="v_pool", bufs=16) as v_pool,
        ):
            with (
                tc.tile_pool(name="ps_sc", bufs=1, space="PSUM") as ps_sc,
                tc.tile_pool(name="ps_av", bufs=2, space="PSUM") as ps_av,
                tc.tile_pool(name="ps_tp", bufs=2, space="PSUM") as ps_tp,
            ):
                sc = [
                    ps_sc.tile([128, 512], F32, tag=f"sc{c}", name=f"sc{c}")
                    for c in range(4)
                ]
                # ALL 8 AV accumulators live as column slices of two PSUM
                # banks -- zero buffer reuse, so no WAR micro-gap between
                # groups ever resets the PE's HAM warm-up window (the gaps
                # kept the whole AV phase clock-gated at 1.2 GHz).
                av_bigs = [
                    ps_av.tile([128, 512], F32, tag="av", name=f"avbig{t}")
                    for t in range(2)
                ]

                # the whole sync-ring stream queues up front in exact
                # consumption order: kt0..15, v0..15 (+newtok column
                # patches), wo0..3, out DMAs. Pool WAR semaphores pace the
                # issue against consumption; nothing else rides sync.
                kt_tiles = {}

                def fetch_kt(u):
                    ktile = kt_pool.tile(
                        [D, 2 * T], FP8, tag="kt", name=f"kt{u}"
                    )
                    nc.sync.dma_start(ktile[:], d["kt"][u])
                    kt_tiles[u] = ktile

                # kt pairs (u, u+8) arrive adjacently: score_pair(u)
                # consumes both as two concurrent col-tiled PE streams.
                # bufs=16 on kt_pool = no WAR gates anywhere in the ring.
                for u in range(8):
                    fetch_kt(u)
                    fetch_kt(u + 8)
                for u in range(B // 2):
                    fetch_v(u)
                for _ in range(4):
                    fetch_wo()

                psum = [
                    small.tile([128, 1], F32, tag=f"psums{c}", name=f"psum{c}")
                    for c in range(4)
                ]
                rs = small.tile([128, 1], F32)
                diag_rs = small.tile([128, 128], BF16)
                # attn^T chunks: pT[t_chunk, (b,h)] for the AV contraction
                pT = big.tile([128, T], BF16, tag="pT")  # free = (c, bh)

                # oT holds attn-out transposed HEAD-major: col 32h + b
                # (contraction chunk h of o_proj = contiguous cols 32h..)
                oT_sb = small.tile([D, B * HQ], BF16)

                pending = []

                def flush_group():
                    av_sb4, fg = pending.pop(0)
                    tp4 = ps_tp.tile(
                        [128, 128], F32, tag="tp", name=f"tpo{fg}"
                    )
                    nc.tensor.transpose(tp4[:], av_sb4[:], eye[:])
                    # tp4 col 32j+h holds (batch 4*fg+j, head h); scatter the
                    # 4x4 used cols into oT[d, 32h + 4fg + j] (cast f32->bf16)
                    # -- one copy per head so every oT write is a contiguous
                    # column slice (plain dst APs for the dep tracker)
                    tp4_r = tp4[:].rearrange("p (j z) -> p j z", j=4)
                    for h in range(HQ):
                        nc.vector.tensor_copy(
                            oT_sb[:, 32 * h + 4 * fg : 32 * h + 4 * fg + 4],
                            tp4_r[:, :, h],
                        )

                def score_pair(u):
                    """Scores for kt tiles u (half 0) and u+8 (half 1) as
                    TWO CONCURRENT col-tiled PE streams: half 0's 64-col
                    qpad lhsT loads array cols 0-63 (tile_position (0,0)),
                    half 1's loads cols 64-127 ((0,64)); each stream pulls
                    its own kt rhs through its own XBUS pair and lands in
                    its own partition range of the shared sc banks. Halves
                    that used to serialize (27.6us) now overlap (~14us)."""
                    ka = kt_tiles.pop(u)
                    kb = kt_tiles.pop(u + 8)
                    for j in range(2):
                        ba = 2 * u + j
                        bb = 2 * (u + 8) + j
                        for c in range(4):
                            nc.tensor.matmul(
                                sc[c][0:64, :],
                                qpad[:, 128 * ba : 128 * ba + 64],
                                ka[:, j * T + 512 * c : j * T + 512 * (c + 1)],
                                start=(ba == 0), stop=(ba == 15),
                                tile_position=(0, 0),
                                skip_group_check=True,
                            )
                            nc.tensor.matmul(
                                sc[c][64:128, :],
                                qpad[:, 128 * bb + 64 : 128 * bb + 128],
                                kb[:, j * T + 512 * c : j * T + 512 * (c + 1)],
                                start=(bb == 16), stop=(bb == 31),
                                tile_position=(0, 64),
                                skip_group_check=True,
                            )

                def newtok_all():
                    # new-token score column in bf16, both halves as
                    # concurrent col-tiled pairs. The host rolled chunk 15's
                    # stream order to [2047, 1920..2046] and zeroed the
                    # stale slot, so t=2047 lives at stream col 1920 =
                    # sc[3] col 384; start=True overwrites the fp8 matmul's
                    # zero contribution there.
                    for b in range(16):
                        bb = b + 16
                        nc.tensor.matmul(
                            sc[3][0:64, 384:385],
                            qpad[:, 128 * b : 128 * b + 64],
                            kT_sb[:, b : b + 1],
                            start=(b == 0), stop=(b == 15),
                            tile_position=(0, 0),
                            skip_group_check=True,
                        )
                        nc.tensor.matmul(
                            sc[3][64:128, 384:385],
                            qpad[:, 128 * bb + 64 : 128 * bb + 128],
                            kT_sb[:, bb : bb + 1],
                            start=(bb == 16), stop=(bb == 31),
                            tile_position=(0, 64),
                            skip_group_check=True,
                        )

                def softmax_all():
                    # merged full-128-row softmax (both halves finish
                    # together now): ACT/DVE cost per op is free-dim bound,
                    # so [128,512] ops cost the same as [64,512] -- half
                    # the chain of the old per-half version. No max
                    # subtraction: |score*INVS| <= ~13 by Cauchy-Schwarz,
                    # far below f32 exp overflow (88).
                    for c in range(4):
                        nc.scalar.activation(
                            attn[:, ts(c, 512)], sc[c][:, :], AF.Exp,
                            scale=INVS, accum_out=psum[c][:],
                        )
                        # PE-warmth keeper: a throwaway matmul gated on
                        # this exp's output (writing into the now-dead sc
                        # bank) fires every ~0.7us through the softmax
                        # chain, so the HAM activity monitor never sees an
                        # idle MID window here -- one re-throttle at this
                        # point put the ENTIRE AV phase at 1.2 GHz
                        # (measured 34us cold, ~15us of loss).
                        nc.tensor.matmul(
                            sc[c][0:64, 0:64],
                            attn[:, c * 512 : c * 512 + 64],
                            attn[:, c * 512 : c * 512 + 64],
                            start=True, stop=True, skip_group_check=True,
                        )
                    nc.vector.tensor_add(psum[0][:], psum[0][:], psum[1][:])
                    nc.vector.tensor_add(psum[2][:], psum[2][:], psum[3][:])
                    nc.vector.tensor_add(sums[:], psum[0][:], psum[2][:])
                    # x SK: folds the fp8 V-cache descale (V stored x24)
                    # into the softmax normalization: pT = probs/SK
                    nc.vector.tensor_scalar_mul(sums[:], sums[:], SK)
                    nc.vector.reciprocal(rs[:], sums[:])
                    # diag(1/sum): folds normalization into the pT matmuls
                    nc.vector.tensor_scalar_mul(
                        diag_rs[:], eye[:], rs[:, 0:1]
                    )
                    nc.tensor.matmul(
                        sc[0][0:64, 0:64], diag_rs[:, 0:64],
                        diag_rs[:, 0:64],
                        start=True, stop=True, skip_group_check=True,
                    )
                    for c in range(NCHUNK):
                        tp = ps_tp.tile([128, 128], F32, tag="tp")
                        nc.tensor.matmul(
                            tp[:], attn[:, ts(c, 128)], diag_rs[:]
                        )
                        nc.vector.tensor_copy(pT[:, ts(c, 128)], tp[:])

                def av_group(g):
                    # Col-tiled AV: 4 batches packed per PSUM tile at
                    # partition bases 0/32/64/96 via tile_position; the 4
                    # streams run concurrently on the PE
                    vt0 = v_tiles.pop(2 * g)
                    vt1 = v_tiles.pop(2 * g + 1)
                    av4 = av_bigs[g // 4][:, ts(g % 4, D)]
                    for c in range(NCHUNK):
                        for j in range(4):
                            b = 4 * g + j
                            vt = vt0 if j < 2 else vt1
                            jj = j % 2
                            nc.tensor.matmul(
                                av4[32 * j : 32 * j + HQ, :],
                                pT[:, c * 128 + HQ * b : c * 128 + HQ * b + HQ],
                                vt[:, jj * T + c * D : jj * T + (c + 1) * D],
                                start=(c == 0), stop=False,
                                tile_position=(0, 32 * j),
                                skip_group_check=True,
                            )
                    # exact-bf16 new-token term: K=1 matmul at partition 0
                    # (pT chunk-15 row 0 = t 2047 prob/SK; v_row0 = v_new*SK)
                    for j in range(4):
                        b = 4 * g + j
                        nc.tensor.matmul(
                            av4[32 * j : 32 * j + HQ, :],
                            pT[0:1, 15 * 128 + HQ * b : 15 * 128 + HQ * b + HQ],
                            v_row0[0:1, b * D : (b + 1) * D],
                            start=False, stop=True,
                            tile_position=(0, 32 * j),
                            skip_group_check=True,
                        )
                    av_sb4 = small.tile(
                        [128, D], F32, tag="av_sb", bufs=3, name=f"avs{g}"
                    )
                    nc.scalar.copy(av_sb4[:], av4[:, :])
                    pending.append((av_sb4, g))
                    # flush one group late: the PE transpose of group g-1
                    # runs behind group g's AV matmuls, so the copy's
                    # cross-engine round-trip never idles the PE
                    if len(pending) > 1:
                        flush_group()

                for u in range(8):
                    score_pair(u)
                newtok_all()
                softmax_all()
                for g in range(8):
                    av_group(g)
                while pending:
                    flush_group()

            # ---------------- o_proj: partial column shard ----------------
            # out_partial[32, 4096] = attn_local[32, 512] @ wo_col_shard.T,
            # accumulated over 4 head chunks (contiguous oT cols as lhsT).
            # 4x col-tiled: two [128,512] PSUM tiles each pack 4 output
            # n-banks at partition bases 0/32/64/96, so 4 n-banks' streams
            # run concurrently (~1.8us instead of 7). h-OUTER so the MM
            # stream starts as soon as wo[0] lands; during the final h
            # sweep each slice's copy + 64KB out-DMA pipeline behind the
            # next slice's matmul.
            with tc.tile_pool(name="ps_o", bufs=2, space="PSUM") as ps_o:
                o_big = [
                    ps_o.tile([128, 512], F32, tag="o", name=f"obig{p}")
                    for p in range(2)
                ]
                for h in range(HQ):
                    for p in range(2):
                        for q in range(4):
                            n = 4 * p + q
                            nc.tensor.matmul(
                                o_big[p][32 * q : 32 * q + B, :],
                                oT_sb[:, 32 * h : 32 * h + 32],
                                wo_sb[h][:, ts(n, 512)],
                                start=(h == 0), stop=(h == HQ - 1),
                                tile_position=(0, 32 * q),
                                skip_group_check=True,
                            )
                            if h == HQ - 1:
                                # bf16 partials (host sums them in f32:
                                # each partial is ~1/8 of the output, so
                                # the rounding adds ~0.07% -- negligible);
                                # out DMAs alternate sync/gpsimd so the 8
                                # issues don't serialize on one ring at
                                # the very tail.
                                o_sb = small.tile(
                                    [B, 512], BF16, tag="osb", bufs=4,
                                    name=f"osb{n}",
                                )
                                sl = o_big[p][32 * q : 32 * q + B, :]
                                if n % 2 == 0:
                                    nc.scalar.copy(o_sb[:], sl)
                                else:
                                    nc.vector.tensor_copy(o_sb[:], sl)
                                eng = nc.sync if n % 2 == 0 else nc.gpsimd
                                eng.dma_start(
                                    out_d[:, ts(n, 512)], o_sb[:]
                                )


def _install_ntff_hook():
    """The agent image's antenv lacks axon_hooks; register an equivalent that
    drives NTFF profiling via ctypes into the injected libaxon_pjrt.so, so
    run_bass_kernel_spmd(trace=True) can capture HW exec times."""
    import types, ctypes, contextlib

    try:
        from antenv.axon_hooks import get_axon_ntff_profile_hook  # noqa: F401
        return  # real one exists
    except ImportError:
        pass
    so_path = "/opt/axon/libaxon_pjrt.so"
    try:
        lib = ctypes.CDLL(so_path)
        if not hasattr(lib, "axon_start_nrt_profile"):
            return
    except OSError:
        return
    lib.axon_start_nrt_profile.argtypes = [
        ctypes.POINTER(ctypes.c_int64), ctypes.c_size_t,
    ]
    lib.axon_start_nrt_profile.restype = ctypes.c_int64
    lib.axon_stop_nrt_profile.argtypes = [ctypes.c_char_p]
    lib.axon_stop_nrt_profile.restype = ctypes.c_int64

    @contextlib.contextmanager
    def _hook(output_dir, device_ids):
        import jax

        jax.devices()
        if device_ids:
            ids = (ctypes.c_int64 * len(device_ids))(*device_ids)
            rc = lib.axon_start_nrt_profile(ids, len(device_ids))
        else:
            rc = lib.axon_start_nrt_profile(None, 0)
        if rc != 0:
            raise RuntimeError(f"axon_start_nrt_profile rc={rc}")
        try:
            yield
        finally:
            n = lib.axon_stop_nrt_profile(str(output_dir).encode())
            print(f"ntff profile: {n} file(s) written to {output_dir}")

    mod = types.ModuleType("antenv.axon_hooks")
    mod.get_axon_ntff_profile_hook = lambda: _hook
    mod.set_axon_ntff_profile_hook = lambda h: None
    sys.modules["antenv.axon_hooks"] = mod


_NC_CACHE = None


def _get_nc():
    global _NC_CACHE
    if _NC_CACHE is None:
        _NC_CACHE = build_nc()
    return _NC_CACHE


def _prep_inputs(x, wq, wk, wv, wo, q_norm_w, k_norm_w, cos, sin,
                 k_cache, v_cache, position_ids):
    x = np.asarray(x, np.float32).reshape(B, HID)
    pids = np.asarray(position_ids).reshape(B).astype(np.int64)
    cos_g = np.asarray(cos, np.float32)[pids]  # [B, D]
    sin_g = np.asarray(sin, np.float32)[pids]
    qw = np.asarray(q_norm_w, np.float32)
    kw = np.asarray(k_norm_w, np.float32)
    perm = (np.arange(D) + D // 2) % D
    sgn = np.where(np.arange(D) < D // 2, -1.0, 1.0).astype(np.float32)
    # 1/sqrt(D) folds into the exp() input scale (INVS); 1/SW undoes the
    # wq fp8 scale after the (scale-invariant) RMSNorm
    cosq1 = cos_g * qw[None, :] / SW
    sinq1 = sgn[None, :] * sin_g * qw[perm][None, :] / SW
    cosq = np.ascontiguousarray(np.tile(cosq1, (1, HQ)))
    sinq = np.ascontiguousarray(np.tile(sinq1, (1, HQ)))
    # k_fin lands at SK x true scale to match the fp8 cache column scale;
    # 1/SW undoes the wk fp8 scale after the scale-invariant RMSNorm
    fk = SK / SW
    cosk = np.ascontiguousarray(cos_g * kw[None, :] * fk)
    sink = np.ascontiguousarray(sgn[None, :] * sin_g * kw[perm][None, :] * fk)

    # xt[p, 32c+b] = x[b, 128c+p]
    xt = np.ascontiguousarray(
        x.T.reshape(KC, D, B).transpose(1, 0, 2).reshape(D, KC * B)
    ).astype(NP_BF16)

    wq = np.asarray(wq, np.float32)
    wk = np.asarray(wk, np.float32)
    wv = np.asarray(wv, np.float32)
    wo = np.asarray(wo, np.float32)
    kc_np = np.asarray(k_cache, np.float32)
    vc_np = np.asarray(v_cache, np.float32)

    in_maps = []
    for i in range(N_CORES):
        m = dict(xt=xt, cosq=cosq, sinq=sinq, cosk=cosk, sink=sink)
        # [g][p][(c n)]: group g holds contraction chunks 4g..4g+4
        wqt = wq[i * NQ : (i + 1) * NQ, :].T.reshape(8, 4, 128, NQ)
        m["wqt"] = (np.ascontiguousarray(wqt.transpose(0, 2, 1, 3)).reshape(
            8, 128, 2048
        ) * SW).astype(NP_FP8)
        wkt = wk[i * D : (i + 1) * D, :].T.reshape(2, 16, 128, D)
        m["wkt"] = (np.ascontiguousarray(wkt.transpose(0, 2, 1, 3)).reshape(
            2, 128, 2048
        ) * SW).astype(NP_FP8)
        wvt = wv[i * D : (i + 1) * D, :].T.reshape(2, 16, 128, D)
        m["wvt"] = np.ascontiguousarray(wvt.transpose(0, 2, 1, 3)).reshape(
            2, 128, 2048
        ).astype(NP_BF16)
        # wot[h][d][o] = wo[o, 512i + 128h + d] (column shard, pre-T)
        wot = wo[:, i * NQ : (i + 1) * NQ].reshape(HID, HQ, D)
        m["wot"] = np.ascontiguousarray(wot.transpose(1, 2, 0)).astype(
            NP_BF16
        )
        # kt[u][d][(j t)] = K^T; v[u][p][(j c e)] with stream slot = 128c+p.
        # Chunk 15's stream order is rolled to [2047, 1920..2046] so the
        # new token t=2047 sits at partition 0 of the V tile (and pT row 0)
        # where the K=1 exact-bf16 newtok matmul can address it; the rolled
        # slot's stale cache values are ZEROED (k and v), so the fp8
        # score/AV matmuls contribute nothing there and no device-side
        # patch DMA is needed.
        perm_t = np.concatenate(
            [np.arange(1920), [2047], np.arange(1920, 2047)]
        )
        kti = kc_np[0, :, :, i, :][:, perm_t, :]   # [B, T, D] (copy)
        kti[:, 1920, :] = 0.0
        kti = kti.transpose(0, 2, 1).reshape(B // 2, 2, D, T)
        m["kt"] = (np.ascontiguousarray(kti.transpose(0, 2, 1, 3)).reshape(
            B // 2, D, 2 * T
        ) * SK).astype(NP_FP8)
        vi = vc_np[0, :, :, i, :][:, perm_t, :]    # [B, T, D] (copy)
        vi[:, 1920, :] = 0.0
        vi = vi.reshape(B // 2, 2, NCHUNK, 128, D)
        m["v"] = (np.ascontiguousarray(vi.transpose(0, 3, 1, 2, 4)).reshape(
            B // 2, 128, 2 * T
        ) * SK).astype(NP_FP8)
        in_maps.append(m)
    return in_maps


def kernel(x, wq, wk, wv, wo, q_norm_w, k_norm_w, cos, sin,
           k_cache, v_cache, position_ids, _trace=False, _trace_cores=None):
    nc = _get_nc()
    if _trace:
        _install_ntff_hook()
    in_maps = _prep_inputs(x, wq, wk, wv, wo, q_norm_w, k_norm_w, cos, sin,
                           k_cache, v_cache, position_ids)
    res = run_bass_kernel_spmd(
        nc, in_maps, core_ids=list(range(N_CORES)),
        trace=_trace, trace_cores=_trace_cores,
    )
    out = np.sum(
        [np.asarray(res.results[i]["out"], np.float32) for i in range(N_CORES)],
        axis=0, dtype=np.float32,
    ).reshape(B, 1, HID)
    if _trace:
        return out, res
    return out

